# revision 6
# baseline (speedup 1.0000x reference)
"""3x3 median blur (replicate borders) on 8 TRN2 NeuronCores.

Input : input_batch (32, 512, 512, 3) float32
Output: (32, 512, 512, 3) float32, per-channel 3x3 median, edge-replicated.

Strategy
--------
Pure data parallel: 4 whole images per core. Per image:
  * Rows are split into top half (rows 0..255) and bottom half (256..511);
    the two halves are element-interleaved into one fp16 stream so every
    1-pixel horizontal shift is an even element offset (4B aligned), which
    keeps the vector engine's fp16 tensor_tensor ops in 2x mode.
  * Columns are processed in 2 blocks of 256 output pixels (+1 px halo).
  * Vertical pass: shared pairwise min/max -> per-column (lo, mid, hi).
  * Horizontal pass: med9 = med3(max3(lo), med3(mid), min3(hi)) with
    sliding-window sharing; 18 vector-engine min/max ops per element total.
  * f32<->fp16 conversion runs on the scalar engine; DMA on sync (HWDGE).

fp16 is safe here: values are in [0, 255), the median is an order
statistic, so the result is an input value rounded to fp16 (rel err
<= ~1e-3, far below any meaningful threshold for this problem).
"""

import numpy as np

import concourse.bass as bass
import concourse.mybir as mybir
from concourse.tile import TileContext
from concourse.vector_clock import ScopedClock
from concourse.bass_utils import run_bass_kernel_spmd

F32 = mybir.dt.float32
F16 = mybir.dt.float16
MIN = mybir.AluOpType.min
MAX = mybir.AluOpType.max

N_CORES = 8
B, H, W, C = 32, 512, 512, 3
WC = W * C                      # 1536 f32 elements per image row
IMGS_PER_CORE = B // N_CORES    # 4
HH = H // 2                     # rows per half (256)
P = 128                         # SBUF partitions
RL = HH // P                    # logical rows per partition (2)
SPX = W // 2 + 2                # stored pixels per column block (258)
XW = SPX * C * 2                # X tile width, fp16 interleaved (1548)
LW = XW - 6                     # sliding-pair width (1542)
OW = XW - 12                    # output width per block, interleaved (1536)
OWH = OW // 2                   # output f32 elems per half-row block (768)
INW = (SPX - 1) * C             # f32 elems loaded per row per block (771)


class _TileContext(TileContext):
    """TileContext whose final drain splits its semaphore waits.

    The stock TileContext attaches every end-of-kernel semaphore wait to a
    single Drain instruction; walrus' CTRL encoding fits only one sync wait
    per instruction, so kernels touching more than one processor fail to
    compile. Carry the waits on a chain of nops (one wait each) instead.
    """

    def _drain_and_barrier(self, tick_clock, wait_clock):
        carrier = self.nc.sync.nop(nofuse=True, hint="drain_wait_carrier")
        wait_clock.add_sem_waits(
            carrier.ins, ScopedClock({None: tick_clock.global_clock})
        )
        si = carrier.ins.sync_info
        waits = list(si.on_wait) if si and si.on_wait else []
        if len(waits) > 1:
            si.on_wait = waits[:1]
            for k in range(1, len(waits)):
                extra = self.nc.sync.nop(nofuse=True, hint=f"dwc{k}")
                extra.ins.sync_info = mybir.SyncInfo(
                    on_wait=[waits[k]], on_update=[]
                )
        self.nc.sync.drain()
        self.nc.all_engine_barrier()
        popped = self.nc._tile_sem_poison_stack.pop()
        assert popped is self._sem_poison
        self.nc.clear_and_free_semaphores(list(self.sems.allocated().values()))
        self.nc.all_engine_barrier()


def _split_multi_waits(nc):
    """Walrus in this toolchain encodes at most ONE sync wait per instruction.

    Tile attaches every needed semaphore wait directly to the consuming
    instruction; hoist all but the last onto standalone EventSemaphore
    instructions on the same engine immediately before it.
    """
    for f in nc.m.functions:
        for b in f.blocks:
            il = b.instructions
            out, changed = [], False
            for inst in il:
                si = inst.sync_info
                waits = list(si.on_wait) if si is not None and si.on_wait else []
                if len(waits) > 1:
                    changed = True
                    for w in waits[:-1]:
                        ev = mybir.InstEventSemaphore(
                            name=f"EVW-{nc.next_id()}",
                            engine=inst.engine,
                            ins=[],
                            outs=[],
                            sync_info=mybir.SyncInfo(on_wait=[w], on_update=[]),
                        )
                        out.append(ev)
                    si.on_wait = waits[-1:]
                out.append(inst)
            if changed:
                b.instructions = out


def _emit_block(nc, pools, x, y, img, s, w):
    """One (image, column-block) pass: output pixels [s, s+w) x 512 rows.

    Narrow first/last blocks (w < 256) shorten the serial pipeline-fill at
    kernel start and the cast+DMA tail after the last vector op.
    """
    tt = nc.vector.tensor_tensor
    XW_b = (w + 2) * C * 2          # X width incl. 1px halo each side (fp16)
    OW_b = w * C * 2                # output width per block, interleaved
    OWH_b = w * C                   # output f32 elems per half-row
    plo = max(s - 1, 0)             # loaded pixel range [plo, phi] (clamped)
    phi = min(s + w, W - 1)
    INW_b = (phi - plo + 1) * C     # f32 elems loaded per row
    c0 = plo * C
    co = s * C
    off = 6 if s == 0 else 0        # X offset of the first loaded pixel

    # ---- DMA in: per half, rows (2p-1 .. 2p+2) clamped at image edges ----
    stag_t = pools["stag"].tile([P, 4, INW], F32, tag="stag", name="stag")[:, :, :INW_b]
    stag_b = pools["stag"].tile([P, 4, INW], F32, tag="stag", name="stag")[:, :, :INW_b]
    xi = x[img]                                        # [H, WC]
    ce = c0 + INW_b
    for j in range(3):  # main rows, regular stride-2 row slices (count = P)
        nc.sync.dma_start(stag_t[:, 1 + j, :], xi[j:j + 2 * P - 1:2, c0:ce])
        nc.sync.dma_start(
            stag_b[:, j, :], xi[HH - 1 + j:HH - 1 + j + 2 * P - 1:2, c0:ce]
        )
    # top halo: row 2p-1 (p>=1); p=0 clamps to row 0
    nc.sync.dma_start(stag_t[1:P, 0, :], xi[1:2 * P - 2:2, c0:ce])
    nc.sync.dma_start(stag_t[0:1, 0, :], xi[0:1, c0:ce])
    # bottom halo: row 256+2p+2 (p<=126); p=127 clamps to row 511
    nc.sync.dma_start(stag_b[0:P - 1, 3, :], xi[HH + 2:H - 1:2, c0:ce])
    nc.sync.dma_start(stag_b[P - 1:P, 3, :], xi[H - 1:H, c0:ce])

    # ---- cast f32 -> fp16, interleaving the two halves ----
    X = pools["x"].tile([P, 4, XW], F16, tag="x", name="x")[:, :, :XW_b]
    nc.scalar.copy(X[:, :, off:off + 2 * INW_b:2], stag_t[:, :, :])
    nc.scalar.copy(X[:, :, off + 1:off + 2 * INW_b:2], stag_b[:, :, :])
    if s == 0:                      # replicate left edge
        nc.scalar.copy(X[:, :, 0:6], X[:, :, 6:12])
    if s + w == W:                  # replicate right edge
        nc.scalar.copy(X[:, :, XW_b - 6:XW_b], X[:, :, XW_b - 12:XW_b - 6])

    # ---- vertical pass: column sort3 with shared pairwise min/max ----
    def wt():
        t = pools["work"].tile([P, RL, XW], F16, tag="work", name="work")
        return t[:, :, :XW_b]

    # output row i's window is X slots (i, i+1, i+2); only pairs (0,1) and
    # (1,2) are consumed, so compute exactly those two pair-rows.
    pmin = pools["pp"].tile([P, RL, XW], F16, tag="pp", name="pp")[:, :, :XW_b]
    pmax = pools["pp"].tile([P, RL, XW], F16, tag="pp", name="pp")[:, :, :XW_b]
    tt(pmin[:], X[:, 0:2, :], X[:, 1:3, :], op=MIN)
    tt(pmax[:], X[:, 0:2, :], X[:, 1:3, :], op=MAX)
    lo, tq, hi, mid = wt(), wt(), wt(), wt()
    tt(lo[:], pmin[:], X[:, 2:4, :], op=MIN)           # min3
    tt(tq[:], pmax[:], X[:, 2:4, :], op=MIN)
    tt(hi[:], pmax[:], X[:, 2:4, :], op=MAX)           # max3
    tt(mid[:], pmin[:], tq[:], op=MAX)                 # med3

    # ---- horizontal pass (all shifts are even element offsets) ----
    # pair results are only consumed at [0:OW_b] (as index k-1 of a window
    # centered at k), so compute exactly that span.
    t1, c1, p_, q_ = wt(), wt(), wt(), wt()
    tt(t1[:, :, 0:OW_b], lo[:, :, 0:OW_b], lo[:, :, 6:6 + OW_b], op=MAX)
    tt(c1[:, :, 0:OW_b], hi[:, :, 0:OW_b], hi[:, :, 6:6 + OW_b], op=MIN)
    tt(p_[:, :, 0:OW_b], mid[:, :, 0:OW_b], mid[:, :, 6:6 + OW_b], op=MIN)
    tt(q_[:, :, 0:OW_b], mid[:, :, 0:OW_b], mid[:, :, 6:6 + OW_b], op=MAX)
    A, Cm, r_ = wt(), wt(), wt()
    tt(A[:, :, 0:OW_b], t1[:, :, 0:OW_b], lo[:, :, 12:XW_b], op=MAX)   # max3
    tt(Cm[:, :, 0:OW_b], c1[:, :, 0:OW_b], hi[:, :, 12:XW_b], op=MIN)  # min3
    tt(r_[:, :, 0:OW_b], q_[:, :, 0:OW_b], mid[:, :, 12:XW_b], op=MIN)
    Bm = wt()
    tt(Bm[:, :, 0:OW_b], p_[:, :, 0:OW_b], r_[:, :, 0:OW_b], op=MAX)   # med3
    s_, u_, v_ = wt(), wt(), wt()
    tt(s_[:, :, 0:OW_b], A[:, :, 0:OW_b], Bm[:, :, 0:OW_b], op=MIN)
    tt(u_[:, :, 0:OW_b], A[:, :, 0:OW_b], Bm[:, :, 0:OW_b], op=MAX)
    tt(v_[:, :, 0:OW_b], u_[:, :, 0:OW_b], Cm[:, :, 0:OW_b], op=MIN)
    O = pools["o"].tile([P, RL, OW], F16, tag="o", name="o")[:, :, :OW_b]
    tt(O[:], s_[:, :, 0:OW_b], v_[:, :, 0:OW_b], op=MAX)               # med9

    # ---- de-interleave cast back to f32 and DMA out ----
    ot = pools["ostag"].tile([P, RL, OWH], F32, tag="ostag", name="ostag")[:, :, :OWH_b]
    ob = pools["ostag"].tile([P, RL, OWH], F32, tag="ostag", name="ostag")[:, :, :OWH_b]
    nc.scalar.copy(ot[:], O[:, :, 0:OW_b:2])
    nc.scalar.copy(ob[:], O[:, :, 1:OW_b:2])
    yt = y[img, 0:HH, :].rearrange("(p i) w -> p i w", i=RL)
    yb = y[img, HH:H, :].rearrange("(p i) w -> p i w", i=RL)
    nc.sync.dma_start(yt[:, :, co:co + OWH_b], ot[:])
    nc.sync.dma_start(yb[:, :, co:co + OWH_b], ob[:])


def _blocks_for(img, n_imgs):
    """Column-block schedule: narrow first block primes the DMA->cast->DVE
    pipeline quickly; narrow last block shrinks the post-DVE tail."""
    first, last = img == 0, img == n_imgs - 1
    if first and last:
        return [(0, 64), (64, 256), (320, 128), (448, 64)]
    if first:
        return [(0, 64), (64, 256), (320, 192)]
    if last:
        return [(0, 256), (256, 192), (448, 64)]
    return [(0, 256), (256, 256)]


def build_median_nc(reps=1, n_imgs=IMGS_PER_CORE):
    """reps>1 repeats the whole job inside one NEFF, one TileContext per rep.

    Each TileContext exit runs an all-engine barrier and clears/frees its
    semaphores, so per-rep semaphore counts restart from zero — in-NEFF
    repetition can't overflow the 16-bit sync wait fields regardless of reps.
    Used by test.py to measure per-rep HW time as a slope between rep counts.
    """
    nc = bass.Bass("TRN2")
    x = nc.dram_tensor("x", [IMGS_PER_CORE, H, WC], F32, kind="ExternalInput")
    y = nc.dram_tensor("out", [IMGS_PER_CORE, H, WC], F32, kind="ExternalOutput")
    from contextlib import ExitStack

    for _ in range(reps):
        with _TileContext(nc) as tc, ExitStack() as es:
            pools = {
                name: es.enter_context(tc.tile_pool(name=name, bufs=bufs))
                for name, bufs in [
                    ("stag", 4), ("x", 2), ("pp", 2), ("work", 8),
                    ("o", 2), ("ostag", 4),
                ]
            }
            for img in range(n_imgs):
                for s, w in _blocks_for(img, n_imgs):
                    _emit_block(nc, pools, x, y, img, s, w)
    _split_multi_waits(nc)
    return nc


_NC_CACHE = {}


def kernel(input_batch: np.ndarray) -> np.ndarray:
    try:  # persistent NEFF cache: skips the walrus compile on repeat runs
        import jax

        jax.config.update("jax_compilation_cache_dir", "/tmp/jax_neff_cache")
        jax.config.update("jax_persistent_cache_min_entry_size_bytes", -1)
        jax.config.update("jax_persistent_cache_min_compile_time_secs", 0)
    except Exception:
        pass
    input_batch = np.asarray(input_batch)
    assert input_batch.shape == (B, H, W, C), input_batch.shape
    xs = np.ascontiguousarray(input_batch.astype(np.float32, copy=False))
    xs = xs.reshape(B, H, WC)
    if "nc" not in _NC_CACHE:
        _NC_CACHE["nc"] = build_median_nc()
    nc = _NC_CACHE["nc"]
    in_maps = [
        {"x": xs[c * IMGS_PER_CORE:(c + 1) * IMGS_PER_CORE]} for c in range(N_CORES)
    ]
    res = run_bass_kernel_spmd(nc, in_maps, core_ids=list(range(N_CORES)))
    out = np.concatenate([res.results[c]["out"] for c in range(N_CORES)], axis=0)
    return out.reshape(B, H, W, C).astype(np.float32, copy=False)



# revision 8
# speedup vs baseline: 3.3451x; 3.3451x over previous
"""3x3 median blur (replicate borders) on 8 TRN2 NeuronCores.

Input : input_batch (32, 512, 512, 3) float32
Output: (32, 512, 512, 3) float32, per-channel 3x3 median, edge-replicated.

Strategy
--------
Pure data parallel: 4 whole images per core. Per image:
  * Rows are split into top half (rows 0..255) and bottom half (256..511);
    the two halves are element-interleaved into one fp16 stream so every
    1-pixel horizontal shift is an even element offset (4B aligned), which
    keeps the vector engine's fp16 tensor_tensor ops in 2x mode.
  * Columns are processed in 2 blocks of 256 output pixels (+1 px halo).
  * Vertical pass: shared pairwise min/max -> per-column (lo, mid, hi).
  * Horizontal pass: med9 = med3(max3(lo), med3(mid), min3(hi)) with
    sliding-window sharing; 18 vector-engine min/max ops per element total.
  * f32<->fp16 conversion runs on the scalar engine; DMA on sync (HWDGE).

fp16 is safe here: values are in [0, 255), the median is an order
statistic, so the result is an input value rounded to fp16 (rel err
<= ~1e-3, far below any meaningful threshold for this problem).
"""

import numpy as np

import concourse.bass as bass
import concourse.mybir as mybir
from concourse.tile import TileContext
from concourse.vector_clock import ScopedClock
from concourse.bass_utils import run_bass_kernel_spmd

F32 = mybir.dt.float32
F16 = mybir.dt.float16
MIN = mybir.AluOpType.min
MAX = mybir.AluOpType.max

N_CORES = 8
B, H, W, C = 32, 512, 512, 3
WC = W * C                      # 1536 f32 elements per image row
IMGS_PER_CORE = B // N_CORES    # 4
HH = H // 2                     # rows per half (256)
P = 128                         # SBUF partitions
RL = HH // P                    # logical rows per partition (2)
SPX = W // 2 + 2                # stored pixels per column block (258)
XW = SPX * C * 2                # X tile width, fp16 interleaved (1548)
LW = XW - 6                     # sliding-pair width (1542)
OW = XW - 12                    # output width per block, interleaved (1536)
OWH = OW // 2                   # output f32 elems per half-row block (768)
INW = SPX * C                   # f32 elems loaded per row per block (<= 774)


class _TileContext(TileContext):
    """TileContext whose final drain splits its semaphore waits.

    The stock TileContext attaches every end-of-kernel semaphore wait to a
    single Drain instruction; walrus' CTRL encoding fits only one sync wait
    per instruction, so kernels touching more than one processor fail to
    compile. Carry the waits on a chain of nops (one wait each) instead.
    """

    def _drain_and_barrier(self, tick_clock, wait_clock):
        carrier = self.nc.sync.nop(nofuse=True, hint="drain_wait_carrier")
        wait_clock.add_sem_waits(
            carrier.ins, ScopedClock({None: tick_clock.global_clock})
        )
        si = carrier.ins.sync_info
        waits = list(si.on_wait) if si and si.on_wait else []
        if len(waits) > 1:
            si.on_wait = waits[:1]
            for k in range(1, len(waits)):
                extra = self.nc.sync.nop(nofuse=True, hint=f"dwc{k}")
                extra.ins.sync_info = mybir.SyncInfo(
                    on_wait=[waits[k]], on_update=[]
                )
        self.nc.sync.drain()
        self.nc.all_engine_barrier()
        popped = self.nc._tile_sem_poison_stack.pop()
        assert popped is self._sem_poison
        self.nc.clear_and_free_semaphores(list(self.sems.allocated().values()))
        self.nc.all_engine_barrier()


def _split_multi_waits(nc):
    """Walrus in this toolchain encodes at most ONE sync wait per instruction.

    Tile attaches every needed semaphore wait directly to the consuming
    instruction; hoist all but the last onto standalone EventSemaphore
    instructions on the same engine immediately before it.
    """
    for f in nc.m.functions:
        for b in f.blocks:
            il = b.instructions
            out, changed = [], False
            for inst in il:
                si = inst.sync_info
                waits = list(si.on_wait) if si is not None and si.on_wait else []
                if len(waits) > 1:
                    changed = True
                    for w in waits[:-1]:
                        ev = mybir.InstEventSemaphore(
                            name=f"EVW-{nc.next_id()}",
                            engine=inst.engine,
                            ins=[],
                            outs=[],
                            sync_info=mybir.SyncInfo(on_wait=[w], on_update=[]),
                        )
                        out.append(ev)
                    si.on_wait = waits[-1:]
                out.append(inst)
            if changed:
                b.instructions = out


def _emit_block(nc, pools, x, y, img, s, w):
    """One (image, column-block) pass: output pixels [s, s+w) x 512 rows.

    Narrow first/last blocks (w < 256) shorten the serial pipeline-fill at
    kernel start and the cast+DMA tail after the last vector op.
    """
    tt = nc.vector.tensor_tensor
    XW_b = (w + 2) * C * 2          # X width incl. 1px halo each side (fp16)
    OW_b = w * C * 2                # output width per block, interleaved
    OWH_b = w * C                   # output f32 elems per half-row
    plo = max(s - 1, 0)             # loaded pixel range [plo, phi] (clamped)
    phi = min(s + w, W - 1)
    INW_b = (phi - plo + 1) * C     # f32 elems loaded per row
    c0 = plo * C
    co = s * C
    off = 6 if s == 0 else 0        # X offset of the first loaded pixel

    # ---- DMA in: per half, rows (2p-1 .. 2p+2) clamped at image edges ----
    stag_t = pools["stag"].tile([P, 4, INW], F32, tag="stag", name="stag")[:, :, :INW_b]
    stag_b = pools["stag"].tile([P, 4, INW], F32, tag="stag", name="stag")[:, :, :INW_b]
    xi = x[img]                                        # [H, WC]
    ce = c0 + INW_b
    for j in range(3):  # main rows, regular stride-2 row slices (count = P)
        nc.sync.dma_start(stag_t[:, 1 + j, :], xi[j:j + 2 * P - 1:2, c0:ce])
        nc.sync.dma_start(
            stag_b[:, j, :], xi[HH - 1 + j:HH - 1 + j + 2 * P - 1:2, c0:ce]
        )
    # top halo: row 2p-1 (p>=1); p=0 clamps to row 0
    nc.sync.dma_start(stag_t[1:P, 0, :], xi[1:2 * P - 2:2, c0:ce])
    nc.sync.dma_start(stag_t[0:1, 0, :], xi[0:1, c0:ce])
    # bottom halo: row 256+2p+2 (p<=126); p=127 clamps to row 511
    nc.sync.dma_start(stag_b[0:P - 1, 3, :], xi[HH + 2:H - 1:2, c0:ce])
    nc.sync.dma_start(stag_b[P - 1:P, 3, :], xi[H - 1:H, c0:ce])

    # ---- cast f32 -> fp16, interleaving the two halves ----
    X = pools["x"].tile([P, 4, XW], F16, tag="x", name="x")[:, :, :XW_b]
    nc.scalar.copy(X[:, :, off:off + 2 * INW_b:2], stag_t[:, :, :])
    nc.scalar.copy(X[:, :, off + 1:off + 2 * INW_b:2], stag_b[:, :, :])
    if s == 0:                      # replicate left edge
        nc.scalar.copy(X[:, :, 0:6], X[:, :, 6:12])
    if s + w == W:                  # replicate right edge
        nc.scalar.copy(X[:, :, XW_b - 6:XW_b], X[:, :, XW_b - 12:XW_b - 6])

    # ---- vertical pass: column sort3 with shared pairwise min/max ----
    def wt():
        t = pools["work"].tile([P, RL, XW], F16, tag="work", name="work")
        return t[:, :, :XW_b]

    # output row i's window is X slots (i, i+1, i+2); only pairs (0,1) and
    # (1,2) are consumed, so compute exactly those two pair-rows.
    pmin = pools["pp"].tile([P, RL, XW], F16, tag="pp", name="pp")[:, :, :XW_b]
    pmax = pools["pp"].tile([P, RL, XW], F16, tag="pp", name="pp")[:, :, :XW_b]
    tt(pmin[:], X[:, 0:2, :], X[:, 1:3, :], op=MIN)
    tt(pmax[:], X[:, 0:2, :], X[:, 1:3, :], op=MAX)
    lo, tq, hi, mid = wt(), wt(), wt(), wt()
    tt(lo[:], pmin[:], X[:, 2:4, :], op=MIN)           # min3
    tt(tq[:], pmax[:], X[:, 2:4, :], op=MIN)
    tt(hi[:], pmax[:], X[:, 2:4, :], op=MAX)           # max3
    tt(mid[:], pmin[:], tq[:], op=MAX)                 # med3

    # ---- horizontal pass (all shifts are even element offsets) ----
    # pair results are only consumed at [0:OW_b] (as index k-1 of a window
    # centered at k), so compute exactly that span.
    t1, c1, p_, q_ = wt(), wt(), wt(), wt()
    tt(t1[:, :, 0:OW_b], lo[:, :, 0:OW_b], lo[:, :, 6:6 + OW_b], op=MAX)
    tt(c1[:, :, 0:OW_b], hi[:, :, 0:OW_b], hi[:, :, 6:6 + OW_b], op=MIN)
    tt(p_[:, :, 0:OW_b], mid[:, :, 0:OW_b], mid[:, :, 6:6 + OW_b], op=MIN)
    tt(q_[:, :, 0:OW_b], mid[:, :, 0:OW_b], mid[:, :, 6:6 + OW_b], op=MAX)
    A, Cm, r_ = wt(), wt(), wt()
    tt(A[:, :, 0:OW_b], t1[:, :, 0:OW_b], lo[:, :, 12:XW_b], op=MAX)   # max3
    tt(Cm[:, :, 0:OW_b], c1[:, :, 0:OW_b], hi[:, :, 12:XW_b], op=MIN)  # min3
    tt(r_[:, :, 0:OW_b], q_[:, :, 0:OW_b], mid[:, :, 12:XW_b], op=MIN)
    Bm = wt()
    tt(Bm[:, :, 0:OW_b], p_[:, :, 0:OW_b], r_[:, :, 0:OW_b], op=MAX)   # med3
    s_, u_, v_ = wt(), wt(), wt()
    tt(s_[:, :, 0:OW_b], A[:, :, 0:OW_b], Bm[:, :, 0:OW_b], op=MIN)
    tt(u_[:, :, 0:OW_b], A[:, :, 0:OW_b], Bm[:, :, 0:OW_b], op=MAX)
    tt(v_[:, :, 0:OW_b], u_[:, :, 0:OW_b], Cm[:, :, 0:OW_b], op=MIN)
    O = pools["o"].tile([P, RL, OW], F16, tag="o", name="o")[:, :, :OW_b]
    tt(O[:], s_[:, :, 0:OW_b], v_[:, :, 0:OW_b], op=MAX)               # med9

    # ---- de-interleave cast back to f32 and DMA out ----
    ot = pools["ostag"].tile([P, RL, OWH], F32, tag="ostag", name="ostag")[:, :, :OWH_b]
    ob = pools["ostag"].tile([P, RL, OWH], F32, tag="ostag", name="ostag")[:, :, :OWH_b]
    nc.scalar.copy(ot[:], O[:, :, 0:OW_b:2])
    nc.scalar.copy(ob[:], O[:, :, 1:OW_b:2])
    yt = y[img, 0:HH, :].rearrange("(p i) w -> p i w", i=RL)
    yb = y[img, HH:H, :].rearrange("(p i) w -> p i w", i=RL)
    nc.sync.dma_start(yt[:, :, co:co + OWH_b], ot[:])
    nc.sync.dma_start(yb[:, :, co:co + OWH_b], ob[:])


def _blocks_for(img, n_imgs):
    """Column-block schedule: narrow first block primes the DMA->cast->DVE
    pipeline quickly; narrow last block shrinks the post-DVE tail."""
    first, last = img == 0, img == n_imgs - 1
    if first and last:
        return [(0, 64), (64, 192), (256, 192), (448, 64)]
    if first:
        return [(0, 64), (64, 192), (256, 256)]
    if last:
        return [(0, 256), (256, 192), (448, 64)]
    return [(0, 256), (256, 256)]


def build_median_nc(reps=1, n_imgs=IMGS_PER_CORE):
    """reps>1 repeats the whole job inside one NEFF, one TileContext per rep.

    Each TileContext exit runs an all-engine barrier and clears/frees its
    semaphores, so per-rep semaphore counts restart from zero — in-NEFF
    repetition can't overflow the 16-bit sync wait fields regardless of reps.
    Used by test.py to measure per-rep HW time as a slope between rep counts.
    """
    nc = bass.Bass("TRN2")
    x = nc.dram_tensor("x", [IMGS_PER_CORE, H, WC], F32, kind="ExternalInput")
    y = nc.dram_tensor("out", [IMGS_PER_CORE, H, WC], F32, kind="ExternalOutput")
    from contextlib import ExitStack

    for _ in range(reps):
        with _TileContext(nc) as tc, ExitStack() as es:
            pools = {
                name: es.enter_context(tc.tile_pool(name=name, bufs=bufs))
                for name, bufs in [
                    ("stag", 4), ("x", 2), ("pp", 2), ("work", 8),
                    ("o", 2), ("ostag", 4),
                ]
            }
            for img in range(n_imgs):
                for s, w in _blocks_for(img, n_imgs):
                    _emit_block(nc, pools, x, y, img, s, w)
    _split_multi_waits(nc)
    return nc


_NC_CACHE = {}


def kernel(input_batch: np.ndarray) -> np.ndarray:
    try:  # persistent NEFF cache: skips the walrus compile on repeat runs
        import jax

        jax.config.update("jax_compilation_cache_dir", "/tmp/jax_neff_cache")
        jax.config.update("jax_persistent_cache_min_entry_size_bytes", -1)
        jax.config.update("jax_persistent_cache_min_compile_time_secs", 0)
    except Exception:
        pass
    input_batch = np.asarray(input_batch)
    assert input_batch.shape == (B, H, W, C), input_batch.shape
    xs = np.ascontiguousarray(input_batch.astype(np.float32, copy=False))
    xs = xs.reshape(B, H, WC)
    if "nc" not in _NC_CACHE:
        _NC_CACHE["nc"] = build_median_nc()
    nc = _NC_CACHE["nc"]
    in_maps = [
        {"x": xs[c * IMGS_PER_CORE:(c + 1) * IMGS_PER_CORE]} for c in range(N_CORES)
    ]
    res = run_bass_kernel_spmd(nc, in_maps, core_ids=list(range(N_CORES)))
    out = np.concatenate([res.results[c]["out"] for c in range(N_CORES)], axis=0)
    return out.reshape(B, H, W, C).astype(np.float32, copy=False)

